# revision 3
# baseline (speedup 1.0000x reference)
"""Trainium2 Bass kernel for ContrastiveVolumeLoss (NT-Xent over sampled volume points).

Strategy (8-way SPMD):
  - Host: transpose each embedding volume to point-major (V, C) and slice into
    8 spatial slabs of (V/8, C); replicate the tiny location tensors.
  - Device: each core computes linear indices from locations, gathers the
    sampled point embeddings that fall in its slab via indirect DMA (out-of-
    slab indices are biased out of range and skipped via bounds_check, leaving
    zero rows), transposes to channel-major, and one AllReduce(add) merges the
    gathered (C, 2N) matrix across cores.
  - Each core then computes 512 of the 4096 similarity rows (its partition-id
    selects the row block via a dynamic slice), exp(sim/T) with fused row sums
    on the scalar engine, extracts the main/partner diagonals with an identity
    mask, forms sum(log(rowsum - diag) - log(e_uv)) for its rows, and a second
    tiny AllReduce produces the final scalar loss on every core.
"""
import sys

for _p in ("/opt/trn_rl_repo", "/root/.axon_site/_ro/trn_rl_repo"):
    if _p not in sys.path:
        sys.path.append(_p)

import numpy as np

import concourse.bass as bass
import concourse.bacc as bacc
import concourse.mybir as mybir
import concourse.tile as tile
from concourse.bass import IndirectOffsetOnAxis
from concourse.bass_utils import run_bass_kernel_spmd
from concourse.masks import make_identity

# Problem constants (hardcoded per contract)
VOL = (64, 128, 128)
V = VOL[0] * VOL[1] * VOL[2]  # 1048576
C = 32
N = 2048                      # sampled points per volume
N2 = 2 * N                    # 4096
NCORES = 8
VS = V // NCORES              # 131072 voxels per slab
TEMP = 0.07
P = 128
NT = N // P                   # 16 point tiles per embedding
NR = N2 // NCORES             # 512 similarity rows per core

F32 = mybir.dt.float32
I32 = mybir.dt.int32
ALU = mybir.AluOpType
ACTF = mybir.ActivationFunctionType
AX = mybir.AxisListType

_NC_CACHE = None


def _build_kernel(tc, nc, e0s, e1s, loc0, loc1, base, loss):
    pid = nc.partition_id()

    with (
        tc.tile_pool(name="const", bufs=1) as constp,
        tc.tile_pool(name="work", bufs=1) as work,
        tc.tile_pool(name="big", bufs=1) as big,
        tc.tile_pool(name="tp", bufs=2, space="PSUM") as tpp,
        tc.tile_pool(name="mm", bufs=4, space="PSUM") as mmp,
        tc.tile_pool(name="exprow", bufs=2) as expp,
        tc.tile_pool(name="small", bufs=2) as smallp,
        tc.tile_pool(name="dram", bufs=1, space="DRAM") as dramp,
    ):
        ident = constp.tile([P, P], F32)
        make_identity(nc, ident[:])
        ones = constp.tile([P, 1], F32)
        nc.vector.memset(ones[:], 1.0)
        bt = constp.tile([P, 1], F32)
        nc.sync.dma_start(bt[:], base.ap())

        # ---- locations -> linear slab-local indices (int32), OOB biased out
        lt0 = work.tile([P, NT * 3], F32, name="lt0")
        nc.sync.dma_start(lt0[:], loc0.ap().rearrange("(p t) c -> p (t c)", p=P))
        lt1 = work.tile([P, NT * 3], F32, name="lt1")
        nc.sync.dma_start(lt1[:], loc1.ap().rearrange("(p t) c -> p (t c)", p=P))

        def make_idx(lt, name):
            v = lt[:].rearrange("p (t c) -> p t c", c=3)
            ind = work.tile([P, NT], F32, name=f"{name}f")
            tmp = work.tile([P, NT], F32, name=f"{name}t")
            nc.vector.tensor_scalar(
                out=ind[:], in0=v[:, :, 0:1], scalar1=float(VOL[1] * VOL[2]),
                scalar2=None, op0=ALU.mult)
            nc.vector.tensor_scalar(
                out=tmp[:], in0=v[:, :, 1:2], scalar1=float(VOL[2]),
                scalar2=None, op0=ALU.mult)
            nc.vector.tensor_tensor(out=ind[:], in0=ind[:], in1=tmp[:], op=ALU.add)
            nc.vector.tensor_tensor(out=ind[:], in0=ind[:], in1=v[:, :, 2:3], op=ALU.add)
            nc.vector.tensor_scalar(
                out=ind[:], in0=ind[:], scalar1=bt[:, 0:1], scalar2=None,
                op0=ALU.subtract)
            # negatives -> +1e9 so the bounds check skips them
            nc.vector.tensor_scalar(
                out=tmp[:], in0=ind[:], scalar1=0.0, scalar2=1e9,
                op0=ALU.is_lt, op1=ALU.mult)
            nc.vector.tensor_tensor(out=ind[:], in0=ind[:], in1=tmp[:], op=ALU.add)
            idx = work.tile([P, NT], I32, name=f"{name}i")
            nc.vector.tensor_copy(out=idx[:], in_=ind[:])
            return idx

        idx0 = make_idx(lt0, "i0")
        idx1 = make_idx(lt1, "i1")

        # ---- masked gather: rows outside this slab stay zero
        g0 = work.tile([P, NT * C], F32, name="g0")
        g1 = work.tile([P, NT * C], F32, name="g1")
        nc.vector.memset(g0[:], 0.0)
        nc.vector.memset(g1[:], 0.0)
        for g, idx, src in ((g0, idx0, e0s), (g1, idx1, e1s)):
            for t in range(NT):
                nc.gpsimd.indirect_dma_start(
                    out=g[:, t * C:(t + 1) * C],
                    out_offset=None,
                    in_=src.ap(),
                    in_offset=IndirectOffsetOnAxis(ap=idx[:, t:t + 1], axis=0),
                    bounds_check=VS - 1,
                    oob_is_err=False,
                )

        # ---- transpose gathered tiles into channel-major gT (C, 2N)
        gT = big.tile([C, N2], F32, name="gT")
        for half, g in enumerate((g0, g1)):
            gdst = gT[:, half * N:(half + 1) * N].rearrange("c (p t) -> c p t", t=NT)
            for t in range(NT):
                pt = tpp.tile([C, P], F32, name="pt")
                nc.tensor.transpose(out=pt[:], in_=g[:, t * C:(t + 1) * C],
                                    identity=ident[:])
                nc.vector.tensor_copy(out=gdst[:, :, t:t + 1], in_=pt[:])

        # ---- AllReduce the gathered embedding matrix across the 8 cores
        gtd = dramp.tile([C, N2], F32, name="gtd")
        gtr = dramp.tile([C, N2], F32, name="gtr", addr_space="Shared")
        nc.sync.dma_start(gtd[:], gT[:])
        nc.gpsimd.collective_compute(
            "AllReduce", ALU.add,
            replica_groups=[list(range(NCORES))],
            ins=[gtd.opt()], outs=[gtr.opt()],
        )
        embT = big.tile([C, N2], F32, name="embT")
        nc.sync.dma_start(embT[:], gtr[:])

        # ---- own 512 rows (dynamic per-core slice)
        ownT = big.tile([C, NR], F32, name="ownT")
        nc.vector.tensor_copy(out=ownT[:], in_=embT[:, bass.ds(pid * NR, NR)])

        # ---- similarity rows, exp, row sums, diagonals, log terms
        contribs = work.tile([P, 4], F32, name="contribs")
        for j in range(4):
            exp_row = expp.tile([P, N2], F32, name="exp_row")
            rsum = smallp.tile([P, 8], F32, name="rsum")
            for m in range(8):
                ps = mmp.tile([P, 512], F32, name="ps")
                nc.tensor.matmul(
                    out=ps[:],
                    lhsT=ownT[:, j * P:(j + 1) * P],
                    rhs=embT[:, m * 512:(m + 1) * 512],
                    start=True, stop=True)
                nc.scalar.activation(
                    out=exp_row[:, m * 512:(m + 1) * 512], in_=ps[:],
                    func=ACTF.Exp, scale=1.0 / TEMP,
                    accum_out=rsum[:, m:m + 1])
            rs = smallp.tile([P, 1], F32, name="rs")
            nc.vector.reduce_sum(out=rs[:], in_=rsum[:], axis=AX.X)

            tmpd = smallp.tile([P, P], F32, name="tmpd")
            dm = smallp.tile([P, 1], F32, name="dm")
            off_m = pid * NR + j * P
            nc.vector.tensor_tensor(
                out=tmpd[:], in0=exp_row[:, bass.ds(off_m, P)], in1=ident[:],
                op=ALU.mult)
            nc.vector.reduce_sum(out=dm[:], in_=tmpd[:], axis=AX.X)

            tmpe = smallp.tile([P, P], F32, name="tmpe")
            de = smallp.tile([P, 1], F32, name="de")
            off_p = ((pid + 4) & 7) * NR + j * P
            nc.vector.tensor_tensor(
                out=tmpe[:], in0=exp_row[:, bass.ds(off_p, P)], in1=ident[:],
                op=ALU.mult)
            nc.vector.reduce_sum(out=de[:], in_=tmpe[:], axis=AX.X)

            snd = smallp.tile([P, 1], F32, name="snd")
            nc.vector.tensor_tensor(out=snd[:], in0=rs[:], in1=dm[:], op=ALU.subtract)
            lns = smallp.tile([P, 1], F32, name="lns")
            lne = smallp.tile([P, 1], F32, name="lne")
            nc.scalar.activation(out=lns[:], in_=snd[:], func=ACTF.Ln)
            nc.scalar.activation(out=lne[:], in_=de[:], func=ACTF.Ln)
            nc.vector.tensor_tensor(
                out=contribs[:, j:j + 1], in0=lns[:], in1=lne[:], op=ALU.subtract)

        # ---- reduce over own rows, then across cores
        csum = work.tile([P, 1], F32, name="csum")
        nc.vector.reduce_sum(out=csum[:], in_=contribs[:], axis=AX.X)
        tot = tpp.tile([1, 1], F32, name="tot")
        nc.tensor.matmul(out=tot[:], lhsT=ones[:], rhs=csum[:], start=True, stop=True)

        lps = work.tile([1, 64], F32, name="lps")
        nc.vector.memset(lps[:], 0.0)
        nc.scalar.mul(out=lps[0:1, 0:1], in_=tot[:], mul=1.0 / N2)
        lpd = dramp.tile([1, 64], F32, name="lpd")
        ltd = dramp.tile([1, 64], F32, name="ltd", addr_space="Shared")
        nc.sync.dma_start(lpd[:], lps[:])
        nc.gpsimd.collective_compute(
            "AllReduce", ALU.add,
            replica_groups=[list(range(NCORES))],
            ins=[lpd.opt()], outs=[ltd.opt()],
        )
        nc.sync.dma_start(loss.ap(), ltd[0:1, 0:1])


def build_program():
    global _NC_CACHE
    if _NC_CACHE is not None:
        return _NC_CACHE
    nc = bacc.Bacc("TRN2", target_bir_lowering=False, debug=False,
                   num_devices=NCORES)
    e0s = nc.dram_tensor("e0s", [VS, C], F32, kind="ExternalInput")
    e1s = nc.dram_tensor("e1s", [VS, C], F32, kind="ExternalInput")
    loc0 = nc.dram_tensor("loc0", [N, 3], F32, kind="ExternalInput")
    loc1 = nc.dram_tensor("loc1", [N, 3], F32, kind="ExternalInput")
    base = nc.dram_tensor("base", [P, 1], F32, kind="ExternalInput")
    loss = nc.dram_tensor("loss", [1, 1], F32, kind="ExternalOutput")
    with tile.TileContext(nc) as tc:
        _build_kernel(tc, nc, e0s, e1s, loc0, loc1, base, loss)
    nc.compile()
    _NC_CACHE = nc
    return nc


def prepare_in_maps(emb_0, emb_1, locations_0, locations_1):
    e0 = np.ascontiguousarray(
        np.asarray(emb_0, np.float32).reshape(C, V).T)  # (V, C)
    e1 = np.ascontiguousarray(
        np.asarray(emb_1, np.float32).reshape(C, V).T)
    l0 = np.ascontiguousarray(np.asarray(locations_0, np.float32).reshape(N, 3))
    l1 = np.ascontiguousarray(np.asarray(locations_1, np.float32).reshape(N, 3))
    in_maps = []
    for k in range(NCORES):
        in_maps.append({
            "e0s": e0[k * VS:(k + 1) * VS],
            "e1s": e1[k * VS:(k + 1) * VS],
            "loc0": l0,
            "loc1": l1,
            "base": np.full((P, 1), np.float32(k * VS), np.float32),
        })
    return in_maps


def kernel(emb_0, emb_1, locations_0, locations_1):
    nc = build_program()
    in_maps = prepare_in_maps(emb_0, emb_1, locations_0, locations_1)
    res = run_bass_kernel_spmd(nc, in_maps, list(range(NCORES)))
    return np.asarray(res.results[0]["loss"], np.float32).reshape(())


# revision 5
# speedup vs baseline: 50.7042x; 50.7042x over previous
"""Trainium2 Bass kernel for ContrastiveVolumeLoss (NT-Xent over sampled volume points).

Strategy (8-way SPMD):
  - Host: transpose each embedding volume to point-major (V, C) and slice into
    8 spatial slabs of (V/8, C); replicate the tiny location tensors.
  - Device: each core computes linear indices from locations, gathers the
    sampled point embeddings that fall in its slab via indirect DMA (out-of-
    slab indices are biased out of range and skipped via bounds_check, leaving
    zero rows), transposes to channel-major, and one AllReduce(add) merges the
    gathered (C, 2N) matrix across cores.
  - Each core then computes 512 of the 4096 similarity rows (its partition-id
    selects the row block via a dynamic slice), exp(sim/T) with fused row sums
    on the scalar engine, extracts the main/partner diagonals with an identity
    mask, forms sum(log(rowsum - diag) - log(e_uv)) for its rows, and a second
    tiny AllReduce produces the final scalar loss on every core.
"""
import sys

for _p in ("/opt/trn_rl_repo", "/root/.axon_site/_ro/trn_rl_repo"):
    if _p not in sys.path:
        sys.path.append(_p)

import numpy as np

import concourse.bass as bass
import concourse.bacc as bacc
import concourse.mybir as mybir
import concourse.tile as tile
from concourse.bass import IndirectOffsetOnAxis
from concourse.bass_utils import run_bass_kernel_spmd
from concourse.masks import make_identity

# Problem constants (hardcoded per contract)
VOL = (64, 128, 128)
V = VOL[0] * VOL[1] * VOL[2]  # 1048576
C = 32
N = 2048                      # sampled points per volume
N2 = 2 * N                    # 4096
NCORES = 8
VS = V // NCORES              # 131072 voxels per slab
TEMP = 0.07
P = 128
NT = N // P                   # 16 point tiles per embedding
NR = N2 // NCORES             # 512 similarity rows per core

F32 = mybir.dt.float32
I32 = mybir.dt.int32
ALU = mybir.AluOpType
ACTF = mybir.ActivationFunctionType
AX = mybir.AxisListType

_NC_CACHE = None


def _build_kernel(tc, nc, e0s, e1s, loc0, loc1, base, loss, reps=1):
    pid = nc.partition_id()
    for _rep in range(reps):
        _build_body(tc, nc, e0s, e1s, loc0, loc1, base, loss, pid)


def _build_body(tc, nc, e0s, e1s, loc0, loc1, base, loss, pid):
    with (
        tc.tile_pool(name="const", bufs=1) as constp,
        tc.tile_pool(name="work", bufs=1) as work,
        tc.tile_pool(name="big", bufs=1) as big,
        tc.tile_pool(name="tp", bufs=2, space="PSUM") as tpp,
        tc.tile_pool(name="mm", bufs=4, space="PSUM") as mmp,
        tc.tile_pool(name="exprow", bufs=2) as expp,
        tc.tile_pool(name="small", bufs=2) as smallp,
        tc.tile_pool(name="dram", bufs=1, space="DRAM") as dramp,
    ):
        ident = constp.tile([P, P], F32)
        make_identity(nc, ident[:])
        ones = constp.tile([P, 1], F32)
        nc.vector.memset(ones[:], 1.0)
        bt = constp.tile([P, 1], F32)
        nc.sync.dma_start(bt[:], base.ap())

        # ---- locations -> linear slab-local indices (int32), OOB biased out
        lt0 = work.tile([P, NT * 3], F32, name="lt0")
        nc.sync.dma_start(lt0[:], loc0.ap().rearrange("(p t) c -> p (t c)", p=P))
        lt1 = work.tile([P, NT * 3], F32, name="lt1")
        nc.sync.dma_start(lt1[:], loc1.ap().rearrange("(p t) c -> p (t c)", p=P))

        def make_idx(lt, name):
            v = lt[:].rearrange("p (t c) -> p t c", c=3)
            ind = work.tile([P, NT], F32, name=f"{name}f")
            tmp = work.tile([P, NT], F32, name=f"{name}t")
            nc.vector.tensor_scalar(
                out=ind[:], in0=v[:, :, 0:1], scalar1=float(VOL[1] * VOL[2]),
                scalar2=None, op0=ALU.mult)
            nc.vector.tensor_scalar(
                out=tmp[:], in0=v[:, :, 1:2], scalar1=float(VOL[2]),
                scalar2=None, op0=ALU.mult)
            nc.vector.tensor_tensor(out=ind[:], in0=ind[:], in1=tmp[:], op=ALU.add)
            nc.vector.tensor_tensor(out=ind[:], in0=ind[:], in1=v[:, :, 2:3], op=ALU.add)
            nc.vector.tensor_scalar(
                out=ind[:], in0=ind[:], scalar1=bt[:, 0:1], scalar2=None,
                op0=ALU.subtract)
            # negatives -> +1e9 so the bounds check skips them
            nc.vector.tensor_scalar(
                out=tmp[:], in0=ind[:], scalar1=0.0, scalar2=1e9,
                op0=ALU.is_lt, op1=ALU.mult)
            nc.vector.tensor_tensor(out=ind[:], in0=ind[:], in1=tmp[:], op=ALU.add)
            idx = work.tile([P, NT], I32, name=f"{name}i")
            nc.vector.tensor_copy(out=idx[:], in_=ind[:])
            return idx

        idx0 = make_idx(lt0, "i0")
        idx1 = make_idx(lt1, "i1")

        # ---- masked gather: rows outside this slab stay zero
        g0 = work.tile([P, NT * C], F32, name="g0")
        g1 = work.tile([P, NT * C], F32, name="g1")
        nc.vector.memset(g0[:], 0.0)
        nc.vector.memset(g1[:], 0.0)
        for g, idx, src in ((g0, idx0, e0s), (g1, idx1, e1s)):
            for t in range(NT):
                nc.gpsimd.indirect_dma_start(
                    out=g[:, t * C:(t + 1) * C],
                    out_offset=None,
                    in_=src.ap(),
                    in_offset=IndirectOffsetOnAxis(ap=idx[:, t:t + 1], axis=0),
                    bounds_check=VS - 1,
                    oob_is_err=False,
                )

        # ---- transpose gathered tiles into channel-major gT (C, 2N)
        gT = big.tile([C, N2], F32, name="gT")
        for half, g in enumerate((g0, g1)):
            gdst = gT[:, half * N:(half + 1) * N].rearrange("c (p t) -> c p t", t=NT)
            for t in range(NT):
                pt = tpp.tile([C, P], F32, name="pt")
                nc.tensor.transpose(out=pt[:], in_=g[:, t * C:(t + 1) * C],
                                    identity=ident[:])
                nc.vector.tensor_copy(out=gdst[:, :, t:t + 1], in_=pt[:])

        # ---- AllReduce the gathered embedding matrix across the 8 cores
        gtd = dramp.tile([C, N2], F32, name="gtd")
        gtr = dramp.tile([C, N2], F32, name="gtr", addr_space="Shared")
        nc.sync.dma_start(gtd[:], gT[:])
        nc.gpsimd.collective_compute(
            "AllReduce", ALU.add,
            replica_groups=[list(range(NCORES))],
            ins=[gtd.opt()], outs=[gtr.opt()],
        )
        embT = big.tile([C, N2], F32, name="embT")
        nc.sync.dma_start(embT[:], gtr[:])

        # ---- own 512 rows (dynamic per-core slice)
        ownT = big.tile([C, NR], F32, name="ownT")
        nc.vector.tensor_copy(out=ownT[:], in_=embT[:, bass.ds(pid * NR, NR)])

        # ---- similarity rows, exp, row sums, diagonals, log terms
        contribs = work.tile([P, 4], F32, name="contribs")
        for j in range(4):
            exp_row = expp.tile([P, N2], F32, name="exp_row")
            rsum = smallp.tile([P, 8], F32, name="rsum")
            for m in range(8):
                ps = mmp.tile([P, 512], F32, name="ps")
                nc.tensor.matmul(
                    out=ps[:],
                    lhsT=ownT[:, j * P:(j + 1) * P],
                    rhs=embT[:, m * 512:(m + 1) * 512],
                    start=True, stop=True)
                nc.scalar.activation(
                    out=exp_row[:, m * 512:(m + 1) * 512], in_=ps[:],
                    func=ACTF.Exp, scale=1.0 / TEMP,
                    accum_out=rsum[:, m:m + 1])
            rs = smallp.tile([P, 1], F32, name="rs")
            nc.vector.reduce_sum(out=rs[:], in_=rsum[:], axis=AX.X)

            tmpd = smallp.tile([P, P], F32, name="tmpd")
            dm = smallp.tile([P, 1], F32, name="dm")
            off_m = pid * NR + j * P
            nc.vector.tensor_tensor(
                out=tmpd[:], in0=exp_row[:, bass.ds(off_m, P)], in1=ident[:],
                op=ALU.mult)
            nc.vector.reduce_sum(out=dm[:], in_=tmpd[:], axis=AX.X)

            tmpe = smallp.tile([P, P], F32, name="tmpe")
            de = smallp.tile([P, 1], F32, name="de")
            off_p = ((pid + 4) & 7) * NR + j * P
            nc.vector.tensor_tensor(
                out=tmpe[:], in0=exp_row[:, bass.ds(off_p, P)], in1=ident[:],
                op=ALU.mult)
            nc.vector.reduce_sum(out=de[:], in_=tmpe[:], axis=AX.X)

            snd = smallp.tile([P, 1], F32, name="snd")
            nc.vector.tensor_tensor(out=snd[:], in0=rs[:], in1=dm[:], op=ALU.subtract)
            lns = smallp.tile([P, 1], F32, name="lns")
            lne = smallp.tile([P, 1], F32, name="lne")
            nc.scalar.activation(out=lns[:], in_=snd[:], func=ACTF.Ln)
            nc.scalar.activation(out=lne[:], in_=de[:], func=ACTF.Ln)
            nc.vector.tensor_tensor(
                out=contribs[:, j:j + 1], in0=lns[:], in1=lne[:], op=ALU.subtract)

        # ---- reduce over own rows, then across cores
        csum = work.tile([P, 1], F32, name="csum")
        nc.vector.reduce_sum(out=csum[:], in_=contribs[:], axis=AX.X)
        tot = tpp.tile([1, 1], F32, name="tot")
        nc.tensor.matmul(out=tot[:], lhsT=ones[:], rhs=csum[:], start=True, stop=True)

        lps = work.tile([1, 64], F32, name="lps")
        nc.vector.memset(lps[:], 0.0)
        nc.scalar.mul(out=lps[0:1, 0:1], in_=tot[:], mul=1.0 / N2)
        lpd = dramp.tile([1, 64], F32, name="lpd")
        ltd = dramp.tile([1, 64], F32, name="ltd", addr_space="Shared")
        nc.sync.dma_start(lpd[:], lps[:])
        nc.gpsimd.collective_compute(
            "AllReduce", ALU.add,
            replica_groups=[list(range(NCORES))],
            ins=[lpd.opt()], outs=[ltd.opt()],
        )
        nc.sync.dma_start(loss.ap(), ltd[0:1, 0:1])


_NC_CACHE_REPS = {}


def build_program(reps=1):
    if reps in _NC_CACHE_REPS:
        return _NC_CACHE_REPS[reps]
    nc = bacc.Bacc("TRN2", target_bir_lowering=False, debug=False,
                   num_devices=NCORES)
    e0s = nc.dram_tensor("e0s", [VS, C], F32, kind="ExternalInput")
    e1s = nc.dram_tensor("e1s", [VS, C], F32, kind="ExternalInput")
    loc0 = nc.dram_tensor("loc0", [N, 3], F32, kind="ExternalInput")
    loc1 = nc.dram_tensor("loc1", [N, 3], F32, kind="ExternalInput")
    base = nc.dram_tensor("base", [P, 1], F32, kind="ExternalInput")
    loss = nc.dram_tensor("loss", [1, 1], F32, kind="ExternalOutput")
    with tile.TileContext(nc) as tc:
        _build_kernel(tc, nc, e0s, e1s, loc0, loc1, base, loss, reps=reps)
    nc.compile()
    _NC_CACHE_REPS[reps] = nc
    return nc


def prepare_in_maps(emb_0, emb_1, locations_0, locations_1):
    e0 = np.ascontiguousarray(
        np.asarray(emb_0, np.float32).reshape(C, V).T)  # (V, C)
    e1 = np.ascontiguousarray(
        np.asarray(emb_1, np.float32).reshape(C, V).T)
    l0 = np.ascontiguousarray(np.asarray(locations_0, np.float32).reshape(N, 3))
    l1 = np.ascontiguousarray(np.asarray(locations_1, np.float32).reshape(N, 3))
    in_maps = []
    for k in range(NCORES):
        in_maps.append({
            "e0s": e0[k * VS:(k + 1) * VS],
            "e1s": e1[k * VS:(k + 1) * VS],
            "loc0": l0,
            "loc1": l1,
            "base": np.full((P, 1), np.float32(k * VS), np.float32),
        })
    return in_maps


def kernel(emb_0, emb_1, locations_0, locations_1):
    nc = build_program()
    in_maps = prepare_in_maps(emb_0, emb_1, locations_0, locations_1)
    res = run_bass_kernel_spmd(nc, in_maps, list(range(NCORES)))
    return np.asarray(res.results[0]["loss"], np.float32).reshape(())
